# revision 72
# baseline (speedup 1.0000x reference)
"""Trainium2 Bass kernel for nn_MultiHeadAttention_81149112090633.

Math (faithful to the quirky reference):
  energy[q,k,n,h] = sum_d query[n,h,q*64+d] * keys[n,h,k*64+d]
  energy masked with -inf where mask[n,h]==0, softmax over the BATCH axis n,
  out[q,k,n,d] = sum_h att[q,k,n,h] * vsum[n,h,d],  vsum = sum_vh values[n,h,vh*64+d]
  final = rows(k,n) x features(q,d) matrix,  Y = X @ w_out.T + b_out

Sharding: data-parallel over batch n (32 per core x 8 cores). The softmax
couples cores only through the per-(q,k,h) denominator S = sum_n exp(...);
S is combined with an on-device AllReduce (1 MB), everything else is local.

Per-core phases:
  P1: per n: 64 tiny matmuls -> energy psum [128,2048] (partition=(h%2)*64+q,
      col=(h//2)*64+k); masking folded into the matmul via an augmented
      65th contraction row. ACT exp (scale=1/8) -> expm bf16; 4 partial-S
      chains on DVE; expm spilled to HBM. vsum is precomputed on the host.
  CC: AllReduce S; transposed readback -> S^T [128,(q,k)] (both h-halves);
      reciprocal -> RSb2 bf16.
  P2 (per nl-pair): transposed readback of expm -> att^T [128=(2n x h-perm),
      (q,k)]; divide by S; pair-block-diagonal einsum2 (lhsT = [vsum_n0 | 0 /
      0 | vsum_n1], K=128) -> psum [128=(n,d), (q,k)]; split-cast into
      resident X^T_hi (fp8 e4m3) and X^T_lo (e5m2) feature-major tiles.
  P3: Y = X @ (W^T 2^6) via fp8 DoubleRow (K=256/instr, 0.5 cyc/row),
      3 passes (hi*hi + lo*hi + hi*lo) accumulated in one psum group;
      ACT copy with scale 2^-6 -> bf16 out. W (hi+lo fp8) streamed once.
      Units (rc,oc) interleaved under P2: 2 units per completed pair.

The h (seq) axis is partition-permuted (evens then odds) in phase 2; host
permutes vsum rows to match.
"""

import os

import numpy as np
import ml_dtypes

N, L, H, D, E = 256, 64, 64, 64, 4096
NCORES = 8
NN = N // NCORES  # 32 batch elements per core
R = NN * 64  # 2048 output rows per core
NEG = -2000.0  # mask bias pre exp-scale (exp((e-2000)/8) == 0)
OCW = 512  # output columns per W chunk
NOC = E // OCW  # 8
NKT = 16  # DoubleRow k-tiles (256 features each)
WSCALE = 64.0  # W pre-scale 2^6 (keeps fp8-hi of W out of subnormals)

# partition p in phase-2 h-layout corresponds to seq position PERM[p]
PERM = np.array([2 * p for p in range(32)] + [2 * p + 1 for p in range(32)])

_PROGRAM_CACHE = {}


def build_program(nn=NN, n_cores=NCORES, use_collective=True):
    import concourse.bass as bass
    import concourse.mybir as mybir
    import concourse.tile as tile
    from concourse import bacc

    f32 = mybir.dt.float32
    bf16 = mybir.dt.bfloat16
    e4 = mybir.dt.float8e4
    e5 = mybir.dt.float8e5
    AF = mybir.ActivationFunctionType
    DR = mybir.MatmulPerfMode.DoubleRow
    npair = nn // 2

    # small dynamic-DMA scratch: every DMA here is a static-AP HWDGE transfer
    nc = bacc.Bacc(
        trn_type="TRN2", num_devices=n_cores, dynamic_dma_scratch_size=2048
    )

    QKT = nc.dram_tensor("qkt", [nn, 65, 2 * E], bf16, kind="ExternalInput").ap()
    VSBD = nc.dram_tensor("vsbd", [npair, 128, 128], bf16, kind="ExternalInput").ap()
    WHI = nc.dram_tensor("whi", [NOC, 128, NKT * 2 * OCW], e4, kind="ExternalInput").ap()
    WLO = nc.dram_tensor("wlo", [NOC, 128, NKT * 2 * OCW], e5, kind="ExternalInput").ap()
    OUT = nc.dram_tensor("out", [R, E], bf16, kind="ExternalOutput").ap()
    EXPM = nc.dram_tensor("expmbuf", [nn, 128, 2048], bf16, kind="Internal").ap()
    CCIN = nc.dram_tensor("ccin", [128, 2048], f32, kind="Internal").ap()
    CCOUT = nc.dram_tensor(
        "ccout", [128, 2048], f32, kind="Internal", addr_space="Shared"
    ).ap()

    with tile.TileContext(nc) as tc:
        with tc.tile_pool(name="persist", bufs=1) as persist:
            RSb2 = persist.tile([128, 4096], bf16, tag="RSb2")

            # ---------------- Phase 1 ----------------
            with (
                tc.tile_pool(name="p1q", bufs=4) as p1q,
                tc.tile_pool(name="p1e", bufs=7) as p1e,
                tc.tile_pool(name="p1s", bufs=1) as p1s,
                tc.tile_pool(name="ps1", bufs=2, space="PSUM") as psp,
            ):
                S = [
                    p1s.tile([128, 2048], f32, tag=f"S{i}", name=f"S{i}")
                    for i in range(4)
                ]
                expms = []
                for nl in range(nn):
                    qka = p1q.tile([65, 2 * E], bf16, tag="qka")
                    nc.sync.dma_start(qka[:], QKT[nl])
                    ps = psp.tile([128, 2048], f32, tag="ps")
                    for h in range(L):
                        par, j = h % 2, h // 2
                        nc.tensor.matmul(
                            ps[par * 64 : par * 64 + 64, j * 64 : j * 64 + 64],
                            qka[:, h * 64 : h * 64 + 64],
                            qka[:, E + h * 64 : E + h * 64 + 64],
                            start=True,
                            stop=True,
                        )
                    expm = p1e.tile([128, 2048], bf16, tag="expm")
                    i = min(nl // 8, 3)  # S[3] covers nl 24..30; expm-31
                    # folds straight into S0 so the tail is one serial add,
                    # column-halved so the S broadcast pipelines behind it
                    if nl < 31:
                        nc.scalar.activation(expm[:], ps[:], AF.Exp, scale=0.125)
                        if nl % 8 == 0:
                            nc.vector.tensor_copy(S[i][:], expm[:])
                        else:
                            nc.vector.tensor_add(S[i][:], S[i][:], expm[:])
                    else:
                        for hh in range(2):
                            m = slice(hh * 1024, hh * 1024 + 1024)
                            nc.scalar.activation(expm[:, m], ps[:, m], AF.Exp, scale=0.125)
                            nc.vector.tensor_add(S[0][:, m], S[0][:, m], expm[:, m])
                            if use_collective:
                                nc.sync.dma_start(CCIN[:, m], S[0][:, m])
                            else:
                                nc.sync.dma_start(CCOUT[:, m], S[0][:, m])
                    expms.append((nl, expm))
                    if nl >= 4:
                        snl, sx = expms.pop(0)
                        nc.scalar.dma_start(EXPM[snl], sx[:])
                    # early combines off the critical tail
                    if nl == 15:
                        nc.vector.tensor_add(S[0][:], S[0][:], S[1][:])
                    if nl == 23:
                        nc.vector.tensor_add(S[0][:], S[0][:], S[2][:])
                    if nl == 30:
                        nc.vector.tensor_add(S[0][:], S[0][:], S[3][:])
                # deferred spill tail drains during the S barrier
                for snl, sx in expms:
                    nc.scalar.dma_start(EXPM[snl], sx[:])

            # ---------------- Phase 2 + 3 ----------------
            with tc.tile_pool(name="xt", bufs=1) as xtp:
                XTH = xtp.tile([128, 32 * R], e4, tag="XTH")
                XTL = xtp.tile([128, 32 * R], e5, tag="XTL")
                xh = XTH.rearrange("p (ci r) -> p ci r", r=R)
                xl = XTL.rearrange("p (ci r) -> p ci r", r=R)

                WQ = NKT * 2 * OCW // 4  # W DMA split: keep chunks ~1.5us so
                # latency-critical barrier DMAs interleave on the DMA engines

                def load_w_half(pool, dram, oc, dt, tname):
                    wt = pool.tile([128, NKT * 2 * OCW], dt, tag=tname, name=tname)
                    for qq in range(4):
                        s = slice(qq * WQ, (qq + 1) * WQ)
                        nc.sync.dma_start(wt[:, s], dram[oc][:, s])
                    return wt.rearrange("p (kt s o) -> p kt s o", kt=NKT, s=2)

                if use_collective:
                    nc.gpsimd.collective_compute(
                        "AllReduce",
                        mybir.AluOpType.add,
                        replica_groups=[list(range(n_cores))],
                        ins=[CCIN[:]],
                        outs=[CCOUT[:]],
                    )
                # (no-collective path: CCOUT written directly at P1 end)

                with (
                    tc.tile_pool(name="p2a", bufs=2) as p2a,
                    tc.tile_pool(name="p2v", bufs=4) as p2v,
                    tc.tile_pool(name="p3y", bufs=2) as p3y,
                    tc.tile_pool(name="ps2", bufs=3, space="PSUM") as ps2p,
                    tc.tile_pool(name="psy", bufs=2, space="PSUM") as psyp,
                ):
                    araws = {}
                    vsbs = {}

                    def fetch_pair(pp):
                        # SP queue: ACT-seq must stay free for the casts.
                        # Column order (qp, t, k) with q = qp + 2t: each
                        # einsum2 chunk then holds one q-parity, halving the
                        # number of strided psum->fp8 cast ops.
                        araw = p2a.tile([128, 4096], bf16, tag="araw", name="araw")
                        for half, nl in ((0, 2 * pp), (1, 2 * pp + 1)):
                            er = EXPM[nl].rearrange(
                                "(par q) (j k) -> par j q k", par=2, k=64
                            )
                            o = half * 64
                            for par in range(2):
                                for qp in range(2):
                                    dst = araw[
                                        o + par * 32 : o + par * 32 + 32,
                                        qp * 2048 : qp * 2048 + 2048,
                                    ].rearrange("p (t k) -> p t k", k=64)
                                    nc.sync.dma_start(dst, er[par][:, qp::2, :])
                        araws[pp] = araw
                        vsb = p2v.tile([128, 128], bf16, tag="vsb", name="vsb")
                        nc.sync.dma_start(vsb[:], VSBD[pp])
                        vsbs[pp] = vsb

                    # W-hi pool opens early; oc0-hi streams during the S window
                    p3wh_cm = tc.tile_pool(name="p3wh", bufs=2)
                    p3wh = p3wh_cm.__enter__()

                    def attmul(pp):
                        # divide by S (att = expm / S) on Pool (gpsimd is the
                        # only engine with slack; it cannot touch PSUM anyway)
                        araw = araws[pp]
                        nc.gpsimd.tensor_mul(
                            araw[:, 0:2048], araw[:, 0:2048], RSb2[:, 0:2048]
                        )
                        nc.gpsimd.tensor_mul(
                            araw[:, 2048:4096], araw[:, 2048:4096], RSb2[:, 2048:4096]
                        )

                    # S^T readback (both h-parity halves stacked twice), recip.
                    # Column-halved so recip h0 starts after 4 DMAs; in-place.
                    # pair-0/1 readbacks queue behind the S^T DMAs so CCIN/CC
                    # stay at the front of the DMA queue at the barrier.
                    with tc.tile_pool(name="pst", bufs=2) as pst:
                        ccr = CCOUT.rearrange("(par q) (j k) -> par j q k", par=2, k=64)
                        # pair-0 readback, criticality-ordered: only the qp0
                        # half gates the first divides; qp1 rides behind the
                        # first S^T half in the DMA queue
                        araw0 = p2a.tile([128, 4096], bf16, tag="araw", name="araw")
                        araws[0] = araw0

                        def fetch0_qp(qp):
                            for half, nl in ((0, 0), (1, 1)):
                                er = EXPM[nl].rearrange(
                                    "(par q) (j k) -> par j q k", par=2, k=64
                                )
                                o = half * 64
                                for par in range(2):
                                    dst = araw0[
                                        o + par * 32 : o + par * 32 + 32,
                                        qp * 2048 : qp * 2048 + 2048,
                                    ].rearrange("p (t k) -> p t k", k=64)
                                    nc.sync.dma_start(dst, er[par][:, qp::2, :])

                        fetch0_qp(0)
                        vsb0 = p2v.tile([128, 128], bf16, tag="vsb", name="vsb")
                        nc.sync.dma_start(vsb0[:], VSBD[0])
                        vsbs[0] = vsb0
                        for ch in range(2):
                            cs = slice(ch * 2048, ch * 2048 + 2048)
                            sth = pst.tile([128, 2048], f32, tag="sth")
                            nc.sync.dma_start(sth[0:32, :], ccr[0][:, ch::2, :])
                            nc.sync.dma_start(sth[32:64, :], ccr[1][:, ch::2, :])
                            nc.sync.dma_start(sth[64:96, :], ccr[0][:, ch::2, :])
                            nc.sync.dma_start(sth[96:128, :], ccr[1][:, ch::2, :])
                            if ch == 0:
                                fetch0_qp(1)
                            nc.vector.reciprocal(sth[:], sth[:])
                            nc.scalar.activation(RSb2[:, cs], sth[:], AF.Copy)
                            if ch == 0:
                                nc.gpsimd.tensor_mul(
                                    araws[0][:, 0:1024],
                                    araws[0][:, 0:1024],
                                    sth[:, 0:1024],
                                )
                                nc.gpsimd.tensor_mul(
                                    araws[0][:, 1024:2048],
                                    araws[0][:, 1024:2048],
                                    sth[:, 1024:2048],
                                )
                            else:
                                nc.gpsimd.tensor_mul(
                                    araws[0][:, 2048:4096],
                                    araws[0][:, 2048:4096],
                                    sth[:],
                                )
                        # pair-1 readback after the S^T chain: it is not
                        # needed until the next slot's divide
                        fetch_pair(1)

                    # W-lo pool opens after the recip scratch frees (SBUF)
                    p3wl_cm = tc.tile_pool(name="p3wl", bufs=2)
                    p3wl = p3wl_cm.__enter__()
                    wA = [
                        (
                            load_w_half(p3wh, WHI, 0, e4, "wh"),
                            load_w_half(p3wl, WLO, 0, e5, "wl"),
                        ),
                        (
                            load_w_half(p3wh, WHI, 1, e4, "wh"),
                            load_w_half(p3wl, WLO, 1, e5, "wl"),
                        ),
                    ]

                    def emit_unit(rc, oc, wh, wl):
                        psY = psyp.tile([128, 512], f32, tag="psY")
                        for oh in range(2):
                            out = psY[:, oh * 256 : oh * 256 + 256]
                            for pi, (xt, wt) in enumerate(((xh, wh), (xl, wh), (xh, wl))):
                                for kt in range(NKT):
                                    nc.tensor.matmul(
                                        out,
                                        xt[:, 2 * kt : 2 * kt + 2, rc * 128 : rc * 128 + 128],
                                        wt[:, kt, :, oh * 256 : oh * 256 + 256],
                                        start=(pi == 0 and kt == 0),
                                        stop=(pi == 2 and kt == NKT - 1),
                                        perf_mode=DR,
                                    )
                        yb = p3y.tile([128, 512], bf16, tag="yb")
                        nc.scalar.activation(yb[:], psY[:], AF.Copy, scale=1.0 / WSCALE)
                        nc.sync.dma_start(
                            OUT[rc * 128 : rc * 128 + 128, oc * OCW : oc * OCW + OCW],
                            yb[:],
                        )

                    def emit_e2_and_casts(pp):
                        nl0, nl1 = 2 * pp, 2 * pp + 1
                        araw = araws.pop(pp)
                        vsb = vsbs.pop(pp)
                        for c in range(4):
                            ps2 = ps2p.tile([128, 1024], f32, tag="ps2")
                            for c2 in range(2):
                                nc.tensor.matmul(
                                    ps2[:, c2 * 512 : c2 * 512 + 512],
                                    vsb[:],
                                    araw[:, c * 1024 + c2 * 512 : c * 1024 + c2 * 512 + 512],
                                    start=True,
                                    stop=True,
                                )
                            v = ps2.rearrange("p (t k) -> p t k", k=64)
                            qp, tlo = c // 2, (c % 2) * 16
                            for np_ in range(2):
                                nl = (nl0, nl1)[np_]
                                src = v[np_ * 64 : np_ * 64 + 64, :, :]
                                dh = xh[
                                    qp * 64 : qp * 64 + 64,
                                    tlo : tlo + 16,
                                    nl * 64 : nl * 64 + 64,
                                ]
                                dl = xl[
                                    qp * 64 : qp * 64 + 64,
                                    tlo : tlo + 16,
                                    nl * 64 : nl * 64 + 64,
                                ]
                                nc.scalar.activation(dh, src, AF.Copy)
                                nc.vector.tensor_sub(dl, src, dh)

                    for pp in range(npair):
                        # next pair's divide runs one slot ahead of its einsum2
                        if pp + 1 < npair:
                            attmul(pp + 1)
                        if pp + 2 < npair:
                            fetch_pair(pp + 2)
                        # einsum2+casts first in PE order: its casts complete
                        # during this slot's units, feeding the next slot
                        emit_e2_and_casts(pp)
                        if pp >= 1:
                            emit_unit(pp - 1, 0, *wA[0])
                            emit_unit(pp - 1, 1, *wA[1])
                    emit_unit(npair - 1, 0, *wA[0])
                    # oc2's W reuses oc0's slot: start its load as soon as the
                    # last oc0 unit has read, under the last oc1 unit
                    wh2 = load_w_half(p3wh, WHI, 2, e4, "wh")
                    emit_unit(npair - 1, 1, *wA[1])
                    wl2 = load_w_half(p3wl, WLO, 2, e5, "wl")

                    # ---------------- Phase 3 tail ----------------
                    for oc in range(2, NOC):
                        if oc == 2:
                            wh, wl = wh2, wl2
                        else:
                            wh = load_w_half(p3wh, WHI, oc, e4, "wh")
                            wl = load_w_half(p3wl, WLO, oc, e5, "wl")
                        for rc in range(R // 128):
                            emit_unit(rc, oc, wh, wl)
                    p3wl_cm.__exit__(None, None, None)
                    p3wh_cm.__exit__(None, None, None)

    nc.compile()
    return nc


def prep_inputs(inputs, nn=NN, n_cores=NCORES):
    """Host-side shard + layout prep. Returns list of per-core input maps."""
    q = np.asarray(inputs["query"], dtype=np.float32)
    k = np.asarray(inputs["keys"], dtype=np.float32)
    v = np.asarray(inputs["values"], dtype=np.float32)
    m = np.asarray(inputs["mask"])
    w = np.asarray(inputs["w_out"], dtype=np.float32)

    # W^T scaled by 2^6, split into e4m3 hi + e5m2 lo, packed per oc chunk:
    # whi[oc, p, kt*1024 + s*512 + o] = W6[(2kt+s)*128 + p, oc*512 + o]
    W6 = np.ascontiguousarray(w.T) * np.float32(WSCALE)
    WHIh = W6.astype(ml_dtypes.float8_e4m3)
    WLOh = (W6 - WHIh.astype(np.float32)).astype(ml_dtypes.float8_e5m2)

    def packw(W8):
        return np.ascontiguousarray(
            W8.reshape(NKT, 2, 128, NOC, OCW)
            .transpose(3, 2, 0, 1, 4)
            .reshape(NOC, 128, NKT * 2 * OCW)
        )

    WHIp, WLOp = packw(WHIh), packw(WLOh)

    vsum = v.reshape(N, L, H, D).sum(axis=2)  # [n, h, d]
    npair = nn // 2

    maps = []
    for c in range(n_cores):
        ns = slice(c * nn, (c + 1) * nn)
        qr = q[ns].reshape(nn, L, H, D)  # [nl, h, qh, d]
        kr = k[ns].reshape(nn, L, H, D)
        QTh = np.empty((nn, 65, L, H), np.float32)
        QTh[:, :64] = qr.transpose(0, 3, 1, 2)  # [nl, d, h, qh]
        QTh[:, 64] = 1.0
        KTh = np.empty((nn, 65, L, H), np.float32)
        KTh[:, :64] = kr.transpose(0, 3, 1, 2)
        KTh[:, 64] = (m[ns].astype(np.float32) - 1.0)[:, :, None] * (-NEG)
        QKh = np.concatenate(
            [QTh.reshape(nn, 65, E), KTh.reshape(nn, 65, E)], axis=2
        ).astype(ml_dtypes.bfloat16)

        vs = vsum[ns][:, PERM, :]  # [nl, h-perm, d]
        vsbd = np.zeros((npair, 128, 128), np.float32)
        for pp in range(npair):
            vsbd[pp, 0:64, 0:64] = vs[2 * pp]
            vsbd[pp, 64:128, 64:128] = vs[2 * pp + 1]

        maps.append(
            {
                "qkt": QKh,
                "vsbd": vsbd.astype(ml_dtypes.bfloat16),
                "whi": WHIp,
                "wlo": WLOp,
            }
        )
    return maps


def assemble_output(core_outs, b_out, nn=NN, n_cores=NCORES):
    """core_outs[c] = [nn*64, E] bf16 with row nl*64+kh -> full (256, 64, E)."""
    n_total = nn * n_cores
    full = np.empty((H, n_total, E), np.float32)  # [kh, n]
    for c in range(n_cores):
        full[:, c * nn : (c + 1) * nn, :] = (
            core_outs[c].astype(np.float32).reshape(nn, H, E).transpose(1, 0, 2)
        )
    full += np.asarray(b_out, dtype=np.float32)
    return full.reshape(n_total, L, E)


def kernel(**inputs) -> np.ndarray:
    from concourse import bass_utils

    key = (NN, NCORES)
    if key not in _PROGRAM_CACHE:
        _PROGRAM_CACHE[key] = build_program(NN, NCORES)
    nc = _PROGRAM_CACHE[key]

    in_maps = prep_inputs(inputs, NN, NCORES)
    trace = bool(int(os.environ.get("KERNEL_TRACE", "0")))
    res = bass_utils.run_bass_kernel_spmd(
        nc,
        in_maps,
        core_ids=list(range(NCORES)),
        trace=trace,
        trace_cores=list(range(NCORES)) if trace else None,
    )
    if trace and res.exec_time_ns is not None:
        print(f"HW exec time: {res.exec_time_ns} ns")
        print(f"HW exec time mean: {res.mean_exec_time_ns} ns")
    core_outs = [r["out"] for r in res.results]
    return assemble_output(core_outs, inputs["b_out"], NN, NCORES)
